# revision 4
# baseline (speedup 1.0000x reference)
"""Trainium2 Bass kernel for SAGAN-style self-attention with spectral-norm 1x1 convs.

Reference computation (per batch element b, with N = H*W = 4096 spatial
positions, C = 256 channels, D = 32 attention dim):
    f = x @ kf + bf ; g = x @ kg + bg ; h = x @ kh + bh      (kX spectrally normalized)
    S = g @ f^T ; beta = softmax(S, axis=-1)
    v = beta @ h ; out = gamma * (v @ kv + bv) + x

Device strategy (data-parallel: one batch element per NeuronCore, 8 cores):
  - Host: power-iteration spectral norm of the tiny weights (fp64), plus folds:
      * bf drops out of softmax entirely (adds a per-query constant to S).
      * bh is folded into the output projection bias: bv' = bh @ kv + bv.
      * gamma is folded into the output projection: kva = [gamma*kv ; gamma*bv'].
  - Device, per core:
      * x^T via DMA xbar transposes straight from HBM bf16.
      * f^T / (g+bg)^T projected into 4 replicated partition groups with
        concurrent column-tiled (tile_position) bf16 matmuls.
      * S^T score tiles via K=32 row-tiled concurrent bf16 matmuls, 3 m-tiles
        ganged per 3-bank PSUM group.
      * exp SPLIT across two engines: ScalarE does true exp on 2 of each
        pack's 3 m-tiles ([128,1024] activation); VectorE does a Schraudolph
        fast-exp on the third (one tensor_scalar: int16(S*128/ln2 + 16250.5)
        bit-viewed as bf16, ~3% elementwise, washes out in softmax).
      * P^T @ [h | 1] with 2x column-tiled (tile_position (0,0)/(0,64))
        concurrent matmuls over even/odd m-tiles, accumulating the attention
        output AND softmax denominators (ones row) in one PSUM bank.
      * epilogue: Z = Za+Zb, 1/Z via one-instruction reciprocal_approx_fast,
        K=1 broadcast matmul, normalize both halves (Za/Z + Zb/Z = 1 keeps
        the folded bias row exact), bf16 output projection, fp32 residual.
      * The PV/epilogue stream lags 3 packs behind the QK/exp stream.
"""

import os
import sys

import numpy as np

try:
    import concourse.bass as bass  # noqa: F401
except Exception:  # pragma: no cover - path setup for fresh environments
    for _p in ("/opt/trn_rl_repo", "/root/.axon_site/_ro/trn_rl_repo"):
        if os.path.isdir(_p) and _p not in sys.path:
            sys.path.insert(0, _p)

B, H, W, C, D = 8, 64, 64, 256, 32
N_FULL = H * W  # 4096

_BUILD_CACHE = {}
LAST_RESULTS = None  # BassKernelResults of the most recent run (for test.py)

# Schraudolph fast-exp constants: bits_bf16(e^x) ~= int16(x * 128/ln2 + c2)
SCH_C1 = 128.0 / float(np.log(2.0))
SCH_C2 = 16250.5


def _l2n64(v):
    return v / np.sqrt(np.maximum((v * v).sum(-1, keepdims=True), 1e-12))


def _sn_kernel_host(w, u):
    """Mirror reference._sn_kernel in float64; returns w / sigma in float32."""
    w64 = np.asarray(w, np.float64)
    u64 = np.asarray(u, np.float64)
    wr = w64.reshape(-1, w64.shape[-1])
    v = _l2n64(u64 @ wr.T)
    u2 = _l2n64(v @ wr)
    sigma = ((v @ wr) @ u2.T)[0, 0]
    return (w64 / sigma).astype(np.float32)


def _build(n, loop_k=1):
    """Build + compile the single-core Bass module for sequence length n.

    loop_k > 1 wraps the whole computation in a hardware loop executing it
    loop_k times — used only for on-device timing (the per-call dispatch
    overhead through the PJRT relay is ~100x the kernel runtime).
    """
    import contextlib

    import concourse.bacc as bacc
    import concourse.mybir as mybir
    import concourse.tile as tile

    f32 = mybir.dt.float32
    f32r = mybir.dt.float32r
    bf16 = mybir.dt.bfloat16
    i16 = mybir.dt.int16
    EXP = mybir.ActivationFunctionType.Exp
    MULT = mybir.AluOpType.mult
    ADD = mybir.AluOpType.add

    NT = n // 128  # number of 128-row tiles (n-tiles == m-tiles)
    NU = n // 512  # number of 512-wide chunks (query blocks)
    packs = []
    m0 = 0
    while m0 < NT:
        sz = min(3, NT - m0)
        packs.append((m0, sz))
        m0 += sz

    nc = bacc.Bacc(
        "TRN2",
        target_bir_lowering=False,
        debug=False,
        enable_asserts=True,
        num_devices=8,
    )
    xb = nc.dram_tensor("xb", [n, C], f32, kind="ExternalInput").ap()
    xbh = nc.dram_tensor("xbh", [n, C], bf16, kind="ExternalInput").ap()
    kf_d = nc.dram_tensor("kf", [C, D], bf16, kind="ExternalInput").ap()
    kg_d = nc.dram_tensor("kg", [C, D], bf16, kind="ExternalInput").ap()
    kh_d = nc.dram_tensor("kh", [C, D], bf16, kind="ExternalInput").ap()
    bg_d = nc.dram_tensor("bg", [D, 1], f32, kind="ExternalInput").ap()
    kva_d = nc.dram_tensor("kva", [D + 1, C], bf16, kind="ExternalInput").ap()
    y = nc.dram_tensor("y", [n, C], f32, kind="ExternalOutput").ap()

    with tile.TileContext(nc) as tc:
        with (
            tc.tile_pool(name="singles", bufs=1) as singles,
            tc.tile_pool(name="ptp", bufs=14) as ptp,
            tc.tile_pool(name="outp", bufs=4) as outp,
            tc.tile_pool(name="smallp", bufs=8) as smallp,
            tc.tile_pool(name="sgp", bufs=2, space="PSUM") as sgp,
            tc.tile_pool(name="opp", bufs=1, space="PSUM") as opp,
            tc.tile_pool(name="oap", bufs=1, space="PSUM") as oap,
        ):
            # ---------------- constants / inputs in SBUF ----------------
            xrows = singles.tile([128, NT, C], f32)  # x rows: [p, tile, c]
            xT = singles.tile([128, 2, n], bf16)  # x^T: [c%128, c//128, n]
            fTr = singles.tile([128, n], bf16)  # f^T replicated on 4 part-groups
            gTr = singles.tile([128, n], bf16)  # (g+bg)^T replicated
            haug = singles.tile([128, NT * 33], bf16)  # [h | 1] per m-tile
            kf_sb = singles.tile([128, 2, D], bf16)
            kg_sb = singles.tile([128, 2, D], bf16)
            kh_sb = singles.tile([128, 2, D], bf16)
            kva_sb = singles.tile([D + 1, C], bf16)
            bgrep = singles.tile([128, 1], f32)
            ones33 = singles.tile([1, D + 1], f32r)

            xb_t = xb.rearrange("(t p) c -> p t c", p=128)
            nc.sync.dma_start(out=kf_sb, in_=kf_d.rearrange("(ch p) d -> p ch d", p=128))
            nc.sync.dma_start(out=kg_sb, in_=kg_d.rearrange("(ch p) d -> p ch d", p=128))
            nc.sync.dma_start(out=kh_sb, in_=kh_d.rearrange("(ch p) d -> p ch d", p=128))
            nc.sync.dma_start(out=kva_sb, in_=kva_d)
            for j in range(4):
                nc.sync.dma_start(out=bgrep[32 * j : 32 * j + 32, :], in_=bg_d)
            ones33_f = singles.tile([1, D + 1], f32)
            nc.gpsimd.memset(ones33_f, 1.0)
            nc.vector.tensor_copy(out=ones33, in_=ones33_f)
            nc.gpsimd.memset(haug, 1.0)

            # ---------------- emission helpers ----------------
            O_tiles = {}

            def emit_chunk(v):
                """x^T, f^T, (g+bg)^T, h for 1024-wide chunk v (m-tiles 8v..8v+7)."""
                sl = slice(1024 * v, 1024 * (v + 1))
                # x^T via DMA xbar transpose straight from HBM (bf16)
                for ch in range(2):
                    nc.sync.dma_start_transpose(
                        out=xT[:, ch, sl],
                        in_=xbh[sl, 128 * ch : 128 * (ch + 1)],
                    )
                # f^T / (g+bg)^T projected into all 4 partition groups via
                # concurrent column-tiled matmuls
                for w_sb, dst, bias in ((kf_sb, fTr, None), (kg_sb, gTr, bgrep)):
                    psR = sgp.tile([128, 1024], f32, tag="sg", name="psR")
                    for h2 in range(2):
                        for ch in range(2):
                            for j in range(4):
                                nc.tensor.matmul(
                                    out=psR[32 * j : 32 * (j + 1), 512 * h2 : 512 * (h2 + 1)],
                                    lhsT=w_sb[:, ch, :],
                                    rhs=xT[:, ch, 1024 * v + 512 * h2 : 1024 * v + 512 * (h2 + 1)],
                                    start=(ch == 0),
                                    stop=(ch == 1),
                                    tile_position=(0, 32 * j),
                                )
                    if bias is None:
                        # f copy on ScalarE (frees DVE time for the exp split)
                        nc.scalar.copy(out=dst[:, sl], in_=psR)
                    else:
                        nc.vector.tensor_scalar_add(out=dst[:, sl], in0=psR, scalar1=bias)
                # h rows for the 8 m-tiles, batched into one PSUM bank
                hp = sgp.tile([128, 8 * D], f32, tag="sg", name="hp")
                for k in range(8):
                    t = 8 * v + k
                    for ch in range(2):
                        nc.tensor.matmul(
                            out=hp[:, D * k : D * (k + 1)],
                            lhsT=xT[:, ch, 128 * t : 128 * (t + 1)],
                            rhs=kh_sb[:, ch, :],
                            start=(ch == 0),
                            stop=(ch == 1),
                        )
                nc.vector.tensor_copy(
                    out=haug[:, 33 * 8 * v : 33 * 8 * (v + 1)].rearrange(
                        "p (k e) -> p k e", k=8
                    )[:, :, 0:D],
                    in_=hp.rearrange("p (k d) -> p k d", k=8),
                )

            def emit_qk_exp(qb, pi):
                p0, sz = packs[pi]
                qs = slice(512 * qb, 512 * (qb + 1))
                sg = sgp.tile([128, 512 * sz], f32, tag="sg", name="sg")
                for i in range(sz):
                    mt = p0 + i
                    nc.tensor.matmul(
                        out=sg[:, 512 * i : 512 * (i + 1)],
                        lhsT=fTr[32 * i : 32 * i + 32, 128 * mt : 128 * (mt + 1)],
                        rhs=gTr[32 * i : 32 * i + 32, qs],
                        start=True,
                        stop=True,
                    )
                pt = ptp.tile([128, 512 * sz], bf16, tag="pt", name="pt")
                # Split the exp: ScalarE true-exp on the first n_act tiles,
                # VectorE Schraudolph bit-trick exp on the rest.
                n_act = 2 if sz == 3 else max(sz - 1, 1)
                nc.scalar.activation(
                    out=pt[:, 0 : 512 * n_act], in_=sg[:, 0 : 512 * n_act], func=EXP
                )
                if sz > n_act:
                    with nc.allow_low_precision(reason="Schraudolph fast-exp bits"):
                        nc.vector.tensor_scalar(
                            out=pt[:, 512 * n_act : 512 * sz].bitcast(i16),
                            in0=sg[:, 512 * n_act : 512 * sz],
                            scalar1=SCH_C1,
                            scalar2=SCH_C2,
                            op0=MULT,
                            op1=ADD,
                        )
                return pt

            def emit_pv(qb, pi, pt):
                p0, sz = packs[pi]
                O = O_tiles[qb]
                for i in range(sz):
                    mt = p0 + i
                    side = mt % 2
                    nc.tensor.matmul(
                        out=O[64 * side : 64 * side + 33, :],
                        lhsT=haug[:, 33 * mt : 33 * mt + 33],
                        rhs=pt[:, 512 * i : 512 * (i + 1)],
                        start=(mt == side),
                        stop=(mt >= NT - 2),
                        tile_position=(0, 64 * side),
                    )

            def emit_epilogue(qb):
                """Z=Za+Zb; 1/Z; normalize both halves; project; residual; store."""
                O = O_tiles[qb]
                vbz = smallp.tile([33, 512], bf16, name="vbz")
                nc.vector.tensor_copy(out=vbz, in_=O[64:97, :])
                Zs = smallp.tile([1, 512], f32, name="Zs")
                nc.vector.tensor_add(out=Zs, in0=O[32:33, :], in1=vbz[32:33, :])
                rs = smallp.tile([1, 512], f32r, name="rs")
                from concourse.dve_ops import (
                    RECIP_APPROX_FAST_CONSTS as _RC,
                    RECIPROCAL_APPROX_FAST as _RF,
                )

                with nc.allow_low_precision(reason="1/Z at ~51 ULP"):
                    nc.vector._custom_dve(
                        _RF, out=rs, in0=Zs, s0=_RC["s0"], s1=_RC["s1"], imm2=_RC["imm2"]
                    )
                rr = opp.tile([D + 1, 512], f32, tag="op", name="rr")
                nc.tensor.matmul(out=rr, lhsT=ones33, rhs=rs, start=True, stop=True)
                rrs = smallp.tile([D + 1, 512], bf16, name="rrs")
                nc.vector.tensor_copy(out=rrs, in_=rr)
                va = smallp.tile([D + 1, 512], bf16, name="va")
                nc.vector.tensor_mul(out=va, in0=O[0:33, :], in1=rrs)
                vb = smallp.tile([D + 1, 512], bf16, name="vb")
                nc.vector.tensor_mul(out=vb, in0=vbz, in1=rrs)
                for j in range(4):
                    nt = 4 * qb + j
                    op_ps = opp.tile([128, C], f32, tag="op", name="op_ps")
                    nc.tensor.matmul(
                        out=op_ps,
                        lhsT=va[:, 128 * j : 128 * (j + 1)],
                        rhs=kva_sb,
                        start=True,
                        stop=False,
                    )
                    nc.tensor.matmul(
                        out=op_ps,
                        lhsT=vb[:, 128 * j : 128 * (j + 1)],
                        rhs=kva_sb,
                        start=False,
                        stop=True,
                    )
                    ot = outp.tile([128, C], f32, name="ot")
                    nc.vector.tensor_add(out=ot, in0=op_ps, in1=xrows[:, nt, :])
                    nc.sync.dma_start(out=y[128 * nt : 128 * (nt + 1), :], in_=ot)

            # ---------------- emission schedule ----------------
            # Prologue chunks interleaved with qb0's packs as their f/h
            # tiles become available, so ScalarE starts exp'ing early.
            loop_cm = (
                tc.For_i(0, loop_k, 1, name="rep")
                if loop_k > 1
                else contextlib.nullcontext()
            )
            loop_cm.__enter__()
            for v in range(NT // 8):
                nc.sync.dma_start(
                    out=xrows[:, 8 * v : 8 * (v + 1), :],
                    in_=xb_t[:, 8 * v : 8 * (v + 1), :],
                )

            # The PV (and epilogue) stream lags LAG packs behind the QK/exp
            # stream: the in-order PE queue then never stalls waiting for an
            # exp, and the engines have lookahead across epilogues.
            # PVs must be popped strictly qb-major (single O accumulator).
            from collections import deque

            LAG = 3 if len(packs) > 3 else 0
            pend = {}
            state = {"outstanding": 0, "cur": 0}

            def push_qk(qb, pi):
                pend.setdefault(qb, deque()).append((pi, emit_qk_exp(qb, pi)))
                state["outstanding"] += 1

            def pop_pv(force=False):
                while state["outstanding"] > (0 if force else LAG):
                    q = state["cur"]
                    if not pend.get(q):
                        break  # current qb has no ready packs yet
                    pi, pt = pend[q].popleft()
                    state["outstanding"] -= 1
                    if pi == 0:
                        O_tiles[q] = oap.tile([128, 512], f32, tag="o", name="O")
                    emit_pv(q, pi, pt)
                    if pi == len(packs) - 1:
                        emit_epilogue(q)
                        state["cur"] += 1
                    if not force:
                        break

            PRO_QBS = min(2, NU)  # qbs interleaved into the prologue
            next_p = [0] * NU
            for v in range(NT // 8):
                emit_chunk(v)
                for qb in range(PRO_QBS):
                    while next_p[qb] < len(packs) and (
                        packs[next_p[qb]][0] + packs[next_p[qb]][1] - 1 <= 8 * v + 7
                    ):
                        push_qk(qb, next_p[qb])
                        next_p[qb] += 1
                        pop_pv()
            for qb in range(NU):
                for pi in range(next_p[qb], len(packs)):
                    push_qk(qb, pi)
                    pop_pv()
            pop_pv(force=True)
            loop_cm.__exit__(None, None, None)

    nc.compile()
    return nc


def _prep_weights(wf, uf, wg, ug, wh, uh, wv, uv, bh, bv, gamma):
    kf = _sn_kernel_host(wf, uf)
    kg = _sn_kernel_host(wg, ug)
    kh = _sn_kernel_host(wh, uh)
    kv = _sn_kernel_host(wv, uv)
    gamma = float(np.asarray(gamma, np.float64)[0])
    bvp = np.asarray(bh, np.float64) @ np.asarray(kv, np.float64) + np.asarray(
        bv, np.float64
    )
    kva = np.concatenate(
        [gamma * np.asarray(kv, np.float64), (gamma * bvp)[None, :]], axis=0
    ).astype(np.float32)
    return kf, kg, kh, kva


def kernel(
    x, wf, bf, uf, wg, bg, ug, wh, bh, uh, wv, bv, uv, gamma, _n=None, _loop_k=1
) -> np.ndarray:
    global LAST_RESULTS
    from concourse import bass_utils

    n = _n or N_FULL
    if (n, _loop_k) not in _BUILD_CACHE:
        _BUILD_CACHE[(n, _loop_k)] = _build(n, loop_k=_loop_k)
    nc = _BUILD_CACHE[(n, _loop_k)]

    import ml_dtypes

    kf, kg, kh, kva = _prep_weights(wf, uf, wg, ug, wh, uh, wv, uv, bh, bv, gamma)
    bg2 = np.ascontiguousarray(np.asarray(bg, np.float32).reshape(D, 1))
    bfd = ml_dtypes.bfloat16
    kf, kg, kh, kva = (np.ascontiguousarray(a.astype(bfd)) for a in (kf, kg, kh, kva))

    x = np.asarray(x, np.float32)
    b = x.shape[0]
    xflat = np.ascontiguousarray(x.reshape(b, -1, C)[:, :n, :])
    xflat_bf = np.ascontiguousarray(xflat.astype(bfd))
    in_maps = [
        {
            "xb": np.ascontiguousarray(xflat[i]),
            "xbh": xflat_bf[i],
            "kf": kf,
            "kg": kg,
            "kh": kh,
            "bg": bg2,
            "kva": kva,
        }
        for i in range(b)
    ]

    trace = bool(int(os.environ.get("BASS_KERNEL_TRACE", "0")))
    try:
        LAST_RESULTS = bass_utils.run_bass_kernel_spmd(
            nc,
            in_maps,
            core_ids=list(range(b)),
            trace=trace,
            trace_cores=[0] if trace else None,
        )
    except ModuleNotFoundError:
        # NTFF profiling hook unavailable in this environment
        LAST_RESULTS = bass_utils.run_bass_kernel_spmd(
            nc, in_maps, core_ids=list(range(b))
        )
    out = np.stack([r["y"] for r in LAST_RESULTS.results], axis=0)
    if n == N_FULL:
        out = out.reshape(b, H, W, C)
    return out
